# revision 45
# baseline (speedup 1.0000x reference)
"""GAT (2-layer, 4-head) forward on 8 Trainium2 NeuronCores (Bass/Tile).

Sharding: destination nodes (and their incident edges, grouped by dst) are
split across 8 cores; each core owns 49 blocks of 128 dst nodes (LPT degree
balanced). Layer-1 node features are computed replicated on every core into a
per-core gather table whose rows are rotated so the core's own nodes occupy
rows [0, 6272).

Row layout (layer L, heads H=4, C feature cols/head): per-head interleaved
[feat(C) | 1.0] x 4, then a_src as f32 bitcast into fp16 slots. The constant
1.0 column rides the bias matmul, so the edge-softmax denominator falls out of
the same one-hot scatter matmul as the weighted feature sum.

Edge aggregation per 128-dst block: dma_gather source rows (int16 indices,
lo/hi base split); per 128-edge chunk a one-hot S (dst-slot is_equal against
an iota row, per-partition scalar = host dst-slot table) scatters messages
into PSUM via matmul. The exp(leaky(e)) edge weight is applied per head with
per-partition-scalar multiplies (4x DVE mode). a_dst reaches edges via
PE-transposed one-hots: ad_e = S^T-matmul against a per-block a_dst column
table kept resident in SBUF. Softmax normalization folds in as a per-(dst,
head) reciprocal of the scattered denominator column. A single AllGather
shares the layer-2 feature table (with baked a_src2 + ones) between cores.
Sigmoid/ELU avoid ACT table swaps (everything uses the Exp/Relu table set).
"""
import sys

sys.path.insert(0, "/opt/trn_rl_repo")

import numpy as np
import ml_dtypes

import concourse.bass as bass
import concourse.mybir as mybir
import concourse.tile as tile
from concourse import bacc
from concourse.bass_utils import run_bass_kernel_spmd

DT2 = mybir.dt.float16
DT8 = mybir.dt.float8e4
F32 = mybir.dt.float32
I16 = mybir.dt.int16
ALU = mybir.AluOpType
ACTF = mybir.ActivationFunctionType

NCORES = 8
P = 128
LO_LIMIT = 32768  # int16 index reach for dma_gather


class Cfg:
    def __init__(self, n_nodes=50000, in_f=256, hid=64, heads=4, labels=32):
        self.N = n_nodes
        self.IN_F = in_f
        self.HID = hid
        self.HEADS = heads
        self.LABELS = labels
        self.D1 = heads * hid
        self.D2 = heads * labels
        self.own = -(-n_nodes // NCORES)
        self.NB = -(-self.own // P)
        self.own_pad = self.NB * P
        self.NPOS = NCORES * self.own_pad
        # layer-1 row: H*(HID+1) msg cols + H a_src cols, fp16, 256B mult
        self.C1 = heads * (hid + 1)                  # 260
        self.W1C = self.C1 + 2 * heads               # 268 psum: +a_s +a_d
        self.ROW1 = -(-(self.C1 + heads) // 128) * 128      # 384 fp16
        self.C2 = heads * (labels + 1)               # 132
        self.W2C = self.C2 + 2 * heads               # 140
        self.ROW2 = -(-(self.C2 + heads) // 128) * 128      # 256 fp16


def _wrap_idx(idx):
    """idx [n] (n%16==0) -> [16, n//16] int16: idx i at [i%16, i//16]."""
    n = len(idx)
    return np.asarray(idx, np.int16).reshape(n // 16, 16).T


class HostPrep:
    def __init__(self, cfg, edge_index):
        self.cfg = cfg
        N = cfg.N
        ei = np.asarray(edge_index, np.int64)
        src = np.concatenate([ei[0], np.arange(N, dtype=np.int64)])
        dst = np.concatenate([ei[1], np.arange(N, dtype=np.int64)])
        deg = np.bincount(dst, minlength=N)

        # LPT: nodes by degree desc -> least-loaded core -> least-loaded block
        order = np.argsort(-deg, kind="stable")
        core_sum = np.zeros(NCORES)
        core_cnt = np.zeros(NCORES, np.int64)
        node_core = np.empty(N, np.int64)
        for n in order:
            c = int(np.argmin(np.where(core_cnt < cfg.own, core_sum, np.inf)))
            node_core[n] = c
            core_sum[c] += deg[n]
            core_cnt[c] += 1
        node_bin = np.empty(N, np.int64)
        node_slot = np.empty(N, np.int64)
        for c in range(NCORES):
            nodes = order[node_core[order] == c]
            bin_sum = np.zeros(cfg.NB)
            bin_cnt = np.zeros(cfg.NB, np.int64)
            for n in nodes:
                b = int(np.argmin(np.where(bin_cnt < P, bin_sum, np.inf)))
                node_bin[n] = b
                node_slot[n] = bin_cnt[b]
                bin_sum[b] += deg[n]
                bin_cnt[b] += 1
        gpos = node_core * cfg.own_pad + node_bin * P + node_slot
        self.node_core, self.node_bin, self.node_slot = node_core, node_bin, node_slot
        self.gpos = gpos
        pos2node = np.full(cfg.NPOS, -1, np.int64)
        pos2node[gpos] = np.arange(N)
        self.pos2node = pos2node

        ecore = node_core[dst]
        ebin = node_bin[dst]
        edslot = node_slot[dst]
        esrc_g = gpos[src]

        self.caps = {1: [np.zeros(cfg.NB, np.int64), np.zeros(cfg.NB, np.int64)],
                     2: [np.zeros(cfg.NB, np.int64), np.zeros(cfg.NB, np.int64)]}
        groups = {}
        for c in range(NCORES):
            m = ecore == c
            sg = esrc_g[m]
            sl = (sg - c * cfg.own_pad) % cfg.NPOS
            bb = ebin[m]
            dl = edslot[m]
            for b in range(cfg.NB):
                mb = bb == b
                for layer, s in ((1, sl[mb]), (2, sg[mb])):
                    lo = s < LO_LIMIT
                    for part, mm in ((0, lo), (1, ~lo)):
                        sv = s[mm] - (LO_LIMIT if part else 0)
                        dv = dl[mb][mm]
                        o = np.argsort(dv, kind="stable")
                        groups[(layer, c, b, part)] = (sv[o], dv[o])
                        self.caps[layer][part][b] = max(
                            self.caps[layer][part][b], -(-len(sv) // P))
        self.groups = groups
        self.maxcap = max(int(self.caps[1][0].max()), int(self.caps[1][1].max()),
                          int(self.caps[2][0].max()), int(self.caps[2][1].max()))

        self.idx_src = {}
        self.dstloc = {}
        for layer in (1, 2):
            lo_cap, hi_cap = self.caps[layer]
            nch = int(lo_cap.sum() + hi_cap.sum())
            for c in range(NCORES):
                iw = np.zeros((16, nch * 8), np.int16)
                dw = np.full((P, nch), 999.0, np.float32)
                col = 0
                ch = 0
                for b in range(cfg.NB):
                    for part in (0, 1):
                        cap = int(self.caps[layer][part][b])
                        if cap == 0:
                            continue
                        sv, dv = groups[(layer, c, b, part)]
                        n = cap * P
                        svp = np.zeros(n, np.int64)
                        dvp = np.full(n, 999, np.int64)
                        svp[:len(sv)] = sv
                        dvp[:len(dv)] = dv
                        iw[:, col:col + n // 16] = _wrap_idx(svp)
                        dw[:, ch:ch + cap] = dvp.reshape(cap, P).T
                        col += n // 16
                        ch += cap
                self.idx_src[(layer, c)] = np.tile(iw, (8, 1))
                self.dstloc[(layer, c)] = dw


def build_program(cfg, prep, with_bias1=True, collective=True):
    nc = bacc.Bacc("TRN2", target_bir_lowering=False, debug=False,
                   num_devices=NCORES)
    H = cfg.HEADS
    HID, LB = cfg.HID, cfg.LABELS
    C1, C2 = cfg.C1, cfg.C2
    NB, NPOS = cfg.NB, cfg.NPOS
    ROW1, ROW2 = cfg.ROW1, cfg.ROW2
    W1C, W2C = cfg.W1C, cfg.W2C
    NT = NPOS // P
    K1 = cfg.IN_F // P
    K2 = cfg.D1 // P

    lo1, hi1 = prep.caps[1]
    lo2, hi2 = prep.caps[2]
    nch1 = int(lo1.sum() + hi1.sum())
    nch2 = int(lo2.sum() + hi2.sum())
    mc = prep.maxcap

    xT = nc.dram_tensor("xT", [cfg.IN_F, NPOS], DT2, kind="ExternalInput")
    w1e = nc.dram_tensor("w1e", [cfg.IN_F, W1C], DT2, kind="ExternalInput")
    w2e = nc.dram_tensor("w2e", [cfg.D1, W2C], DT2, kind="ExternalInput")
    bias1 = nc.dram_tensor("bias1", [1, W1C], DT2, kind="ExternalInput")
    bias2 = nc.dram_tensor("bias2", [1, W2C], DT2, kind="ExternalInput")
    ones1 = nc.dram_tensor("ones1", [1, P], DT2, kind="ExternalInput")
    iota16 = nc.dram_tensor("iota16", [P, P], DT2, kind="ExternalInput")
    ident = nc.dram_tensor("ident", [P, P], DT2, kind="ExternalInput")
    is1 = nc.dram_tensor("is1", [P, nch1 * 8], I16, kind="ExternalInput")
    dl1 = nc.dram_tensor("dl1", [P, nch1], F32, kind="ExternalInput")
    is2 = nc.dram_tensor("is2", [P, nch2 * 8], I16, kind="ExternalInput")
    dl2 = nc.dram_tensor("dl2", [P, nch2], F32, kind="ExternalInput")
    out = nc.dram_tensor("out", [cfg.own_pad, cfg.D2], F32,
                         kind="ExternalOutput")

    with tile.TileContext(nc) as tc:
        with tc.tile_pool(name="dram", bufs=1, space="DRAM") as dram, \
             tc.tile_pool(name="const", bufs=1) as cp:
            table1 = dram.tile([NPOS, ROW1], DT2)
            h2shard = dram.tile([cfg.own_pad, ROW2], DT2)
            table2 = dram.tile([NPOS, ROW2], DT2,
                               addr_space="Shared" if collective else "Local")

            def load_const(name, dram_t, shape, dt):
                t = cp.tile(shape, dt, tag=name, name=name + "_sb")
                nc.sync.dma_start(t[:], dram_t[:])
                return t

            iota16_sb = load_const("iota16", iota16, [P, P], DT2)
            ident_sb = load_const("ident", ident, [P, P], DT2)
            bias1_sb = load_const("bias1", bias1, [1, W1C], DT2)
            bias2_sb = load_const("bias2", bias2, [1, W2C], DT2)
            ones1_sb = load_const("ones1", ones1, [1, P], DT2)
            is1_sb = load_const("is1", is1, [P, nch1 * 8], I16)
            dl1_sb = load_const("dl1", dl1, [P, nch1], F32)
            is2_sb = load_const("is2", is2, [P, nch2 * 8], I16)
            dl2_sb = load_const("dl2", dl2, [P, nch2], F32)
            w1_sb = [cp.tile([P, W1C], DT2, tag=f"w1_{k}", name=f"w1sb{k}")
                     for k in range(K1)]
            for k in range(K1):
                nc.sync.dma_start(w1_sb[k][:], w1e[k * P:(k + 1) * P, :])
            w2_sb = [cp.tile([P, W2C], DT2, tag=f"w2_{k}", name=f"w2sb{k}")
                     for k in range(K2)]
            for k in range(K2):
                nc.sync.dma_start(w2_sb[k][:], w2e[k * P:(k + 1) * P, :])
            # per-block a_dst column tables (fp16), resident all kernel long
            adblk1 = cp.tile([P, NB * H], DT2, tag="adblk1", name="adblk1")
            adblk2 = cp.tile([P, NB * H], DT2, tag="adblk2", name="adblk2")

            # ---------------- Phase A: dense layer 1 (replicated) -----------
            SEG = 16
            TB = 4  # tiles per table1 write batch
            C1R = C1 + H  # row cols: messages + a_src, all fp16
            with tc.tile_pool(name="dA", bufs=3) as dp, \
                 tc.tile_pool(name="dAp", bufs=4, space="PSUM") as dpp:
                for seg in range(0, NT, SEG):
                    ntile = min(SEG, NT - seg)
                    xs = [dp.tile([P, ntile * P], DT2, tag=f"xs{k}", name=f"xs{k}")
                          for k in range(K1)]
                    for k in range(K1):
                        nc.sync.dma_start(
                            xs[k][:],
                            xT[k * P:(k + 1) * P, seg * P:(seg + ntile) * P])
                    for tb in range(0, ntile, TB):
                        nb = min(TB, ntile - tb)
                        rows = dp.tile([P, TB, C1R], DT2, tag="rows")
                        for t in range(tb, tb + nb):
                            ps = dpp.tile([P, W1C], F32, tag="ps")
                            for k in range(K1):
                                nc.tensor.matmul(
                                    ps[:], xs[k][:, t * P:(t + 1) * P],
                                    w1_sb[k][:], start=(k == 0), stop=False)
                            nc.tensor.matmul(ps[:], ones1_sb[:], bias1_sb[:],
                                             start=False, stop=True)
                            # row: [4x(64 h | 1.0) | a_s x4], all fp16
                            gt = seg + t
                            if gt % 2 == 0:
                                nc.vector.tensor_copy(rows[:, t - tb, :],
                                                      ps[:, 0:C1R])
                            else:
                                nc.scalar.copy(rows[:, t - tb, :],
                                               ps[:, 0:C1R])
                            if gt < NB:
                                nc.vector.tensor_copy(
                                    adblk1[:, gt * H:(gt + 1) * H],
                                    ps[:, C1 + H:C1 + 2 * H])
                        gt0 = seg + tb
                        nc.sync.dma_start(
                            table1[gt0 * P:(gt0 + nb) * P, 0:C1R].rearrange(
                                "(t p) c -> p t c", p=P),
                            rows[:, 0:nb, :])

            def edge_phase(layer, tbl_full, caps_lo, caps_hi, is_sb, dl_sb,
                           adblk, CC, row_cols, gdt, epilogue):
                """Shared edge-aggregation phase, software-pipelined over
                (block, lo/hi) parts so cross-engine latency is hidden.
                CC = H*(C+1) message cols."""
                CH = CC // H  # per-head cols incl the ones column
                parts = []
                col = 0
                ch = 0
                for b in range(NB):
                    nchunks = int(caps_lo[b] + caps_hi[b])
                    done0 = 0
                    for part in (0, 1):
                        cap = int((caps_lo if part == 0 else caps_hi)[b])
                        if cap == 0:
                            continue
                        parts.append(dict(
                            b=b, part=part, cap=cap, col=col, ch=ch,
                            first=(done0 == 0),
                            last=(done0 + cap == nchunks),
                            done0=done0, nchunks=nchunks))
                        col += cap * P // 16
                        ch += cap
                        done0 += cap
                nparts = len(parts)
                st = [None] * nparts
                cur_psm = [None]

                with tc.tile_pool(name=f"E{layer}", bufs=2) as bp, \
                     tc.tile_pool(name=f"E{layer}s", bufs=4 * mc + 8) as sp, \
                     tc.tile_pool(name=f"E{layer}p", bufs=2, space="PSUM") as pp, \
                     tc.tile_pool(name=f"E{layer}pt", bufs=2, space="PSUM") as ppt, \
                     tc.tile_pool(name=f"E{layer}pa", bufs=2, space="PSUM") as ppa, \
                     tc.tile_pool(name=f"E{layer}pd", bufs=1, space="PSUM") as ppd:

                    def stageX(i):
                        pr = parts[i]
                        b, cap, pcol, pch = pr['b'], pr['cap'], pr['col'], pr['ch']
                        num = cap * P
                        tbl = (tbl_full[:] if pr['part'] == 0
                               else tbl_full[LO_LIMIT:, :])
                        g = bp.tile([P, cap, row_cols], gdt, tag="g", bufs=3)
                        nc.gpsimd.dma_gather(
                            g[:], tbl, is_sb[:, pcol:pcol + num // 16],
                            num, num, row_cols, single_packet=False)
                        s01s = []
                        adps = ppa.tile([P, cap, H], F32, tag="adps")
                        for jb in range(0, cap, 4):
                            nb4 = min(4, cap - jb)
                            stt = ppt.tile([P, 4, P], DT2, tag="stt")
                            for j in range(jb, jb + nb4):
                                s01 = sp.tile([P, P], DT2, tag="s01")
                                nc.vector.tensor_scalar(
                                    out=s01[:], in0=iota16_sb[:],
                                    scalar1=dl_sb[:, pch + j:pch + j + 1],
                                    scalar2=None, op0=ALU.is_equal)
                                s01s.append(s01)
                                nc.tensor.transpose(
                                    stt[:, j - jb, :], s01[:], ident_sb[:])
                            stts = bp.tile([P, 4, P], DT2, tag="stts", bufs=3)
                            nc.scalar.copy(stts[:, 0:nb4, :], stt[:, 0:nb4, :])
                            for j in range(jb, jb + nb4):
                                nc.tensor.matmul(
                                    adps[:, j, :], stts[:, j - jb, :],
                                    adblk[:, b * H:(b + 1) * H],
                                    start=True, stop=True)
                        st[i] = [g, s01s, adps, None]

                    def stageY1(i):
                        g, s01s, adps, _ = st[i]
                        cap = parts[i]['cap']
                        as_e = g[:, :, CC:CC + H]
                        lg = bp.tile([P, cap, H], F32, tag="lg", bufs=3)
                        nc.vector.tensor_tensor(
                            out=lg[:], in0=as_e, in1=adps[:], op=ALU.add)
                        # leaky = max(0.2*lg, lg), then exp
                        nc.vector.scalar_tensor_tensor(
                            out=lg[:], in0=lg[:], scalar=0.2, in1=lg[:],
                            op0=ALU.mult, op1=ALU.max)
                        e1 = bp.tile([P, cap, H], F32, tag="e1", bufs=3)
                        nc.scalar.activation(e1[:], lg[:], ACTF.Exp)
                        st[i][3] = e1

                    def stageY2(i):
                        pr = parts[i]
                        g, s01s, adps, e1 = st[i]
                        cap = pr['cap']
                        if pr['first']:
                            cur_psm[0] = pp.tile([P, CC], F32, tag="psm",
                                                 name="psm")
                        psm = cur_psm[0]
                        rs = bp.tile([P, cap, H, CH], DT2, tag="rs", bufs=3)
                        gv = g[:, :, 0:CC].rearrange("p a (h c) -> p a h c",
                                                     h=H)
                        nc.vector.tensor_tensor(
                            out=rs[:],
                            in0=gv[:],
                            in1=e1[:, :, :, None].broadcast_to(
                                [P, cap, H, CH]),
                            op=ALU.mult)
                        for j in range(cap):
                            nc.tensor.matmul(
                                psm[:],
                                s01s[j][:],
                                rs[:, j, :, :].rearrange("p a b -> p (a b)"),
                                start=(pr['done0'] + j == 0),
                                stop=(pr['done0'] + j == pr['nchunks'] - 1))
                        st[i] = None
                        return psm if pr['last'] else None

                    pending = None
                    for i in range(min(2, nparts)):
                        stageX(i)
                    for i in range(nparts):
                        stageY1(i)
                        if i + 2 < nparts:
                            stageX(i + 2)
                        if pending is not None:
                            epilogue(pending[0], pending[1], bp, ppd)
                            pending = None
                        r = stageY2(i)
                        if r is not None:
                            pending = (parts[i]['b'], r)
                    if pending is not None:
                        epilogue(pending[0], pending[1], bp, ppd)
                return

            # ------- Phase B: layer-1 edges + layer-2 dense, per block ------
            def epilogue1(b, psm, bp, ppd):
                H_, HID_ = H, HID
                dn = bp.tile([P, H_], F32, tag="dn")
                nc.vector.tensor_scalar(
                    out=dn[:], in0=psm[:, HID_:C1:HID_ + 1], scalar1=1e-16,
                    scalar2=None, op0=ALU.add)
                rc = bp.tile([P, H_], F32, tag="rc")
                nc.vector.reciprocal(rc[:], dn[:])
                o1 = bp.tile([P, cfg.D1], F32, tag="o1")
                nc.vector.tensor_tensor(
                    out=o1[:].rearrange("p (h d) -> p h d", h=H_),
                    in0=psm[:].rearrange("p (h x) -> p h x", h=H_)[:, :, 0:HID_],
                    in1=rc[:][:, :, None].broadcast_to([P, H_, HID_]),
                    op=ALU.mult)
                # sfull = ELU(o1)+1 = relu(o1) + exp(-relu(-o1))
                r1 = bp.tile([P, cfg.D1], F32, tag="r1")
                nc.scalar.activation(r1[:], o1[:], ACTF.Relu)
                n1 = bp.tile([P, cfg.D1], F32, tag="n1")
                nc.scalar.activation(n1[:], o1[:], ACTF.Relu, scale=-1.0)
                e3 = bp.tile([P, cfg.D1], F32, tag="e3")
                nc.scalar.activation(e3[:], n1[:], ACTF.Exp, scale=-1.0)
                sfull = bp.tile([P, cfg.D1], DT2, tag="sfull")
                nc.vector.tensor_tensor(
                    out=sfull[:], in0=r1[:], in1=e3[:], op=ALU.add)
                ps2 = ppd.tile([P, W2C], F32, tag="ps2")
                for k in range(K2):
                    pt = ppd.tile([P, P], DT2, tag="pt")
                    nc.tensor.transpose(
                        pt[:], sfull[:, k * P:(k + 1) * P], ident_sb[:])
                    st = bp.tile([P, P], DT2, tag="st")
                    nc.vector.tensor_copy(st[:], pt[:])
                    nc.tensor.matmul(ps2[:], st[:], w2_sb[k][:],
                                     start=(k == 0), stop=False)
                nc.tensor.matmul(ps2[:], ones1_sb[:], bias2_sb[:],
                                 start=False, stop=True)
                h2r = bp.tile([P, C2 + H], DT2, tag="h2r")
                nc.scalar.copy(h2r[:], ps2[:, 0:C2 + H])
                nc.vector.tensor_copy(
                    adblk2[:, b * H:(b + 1) * H], ps2[:, C2 + H:C2 + 2 * H])
                nc.sync.dma_start(
                    h2shard[b * P:(b + 1) * P, 0:C2 + H], h2r[:])

            edge_phase(1, table1, lo1, hi1, is1_sb, dl1_sb, adblk1, C1, ROW1,
                       DT2, epilogue1)

            # ---------------- Phase C: AllGather ----------------------------
            if collective:
                nc.gpsimd.collective_compute(
                    "AllGather", ALU.bypass,
                    replica_groups=[list(range(NCORES))],
                    ins=[h2shard[:].opt()],
                    outs=[table2[:].opt()],
                )
            else:  # timing-only variant: fake the exchange with a local copy
                for r in range(NCORES):
                    nc.sync.dma_start(
                        table2[r * cfg.own_pad:(r + 1) * cfg.own_pad, :],
                        h2shard[:])

            # ---------------- Phase D: layer-2 edges ------------------------
            def epilogue2(b, psm, bp, ppd):
                H_, LB_ = H, LB
                dn = bp.tile([P, H_], F32, tag="dn")
                nc.vector.tensor_scalar(
                    out=dn[:], in0=psm[:, LB_:C2:LB_ + 1], scalar1=1e-16,
                    scalar2=None, op0=ALU.add)
                rc = bp.tile([P, H_], F32, tag="rc")
                nc.vector.reciprocal(rc[:], dn[:])
                o2 = bp.tile([P, cfg.D2], F32, tag="o2")
                nc.vector.tensor_tensor(
                    out=o2[:].rearrange("p (h d) -> p h d", h=H_),
                    in0=psm[:].rearrange("p (h x) -> p h x", h=H_)[:, :, 0:LB_],
                    in1=rc[:][:, :, None].broadcast_to([P, H_, LB_]),
                    op=ALU.mult)
                # sigmoid(x) = 0.5*tanh(x/2) + 0.5 (Tanh shares the Exp table)
                sg = bp.tile([P, cfg.D2], F32, tag="sg")
                nc.scalar.activation(sg[:], o2[:], ACTF.Tanh, scale=0.5)
                ov = bp.tile([P, cfg.D2], F32, tag="ov")
                nc.vector.tensor_scalar(
                    out=ov[:], in0=sg[:], scalar1=0.5, scalar2=0.5,
                    op0=ALU.mult, op1=ALU.add)
                nc.sync.dma_start(out[b * P:(b + 1) * P, :], ov[:])

            edge_phase(2, table2, lo2, hi2, is2_sb, dl2_sb, adblk2, C2, ROW2,
                       DT2, epilogue2)

    nc.compile()
    return nc


def make_inputs(cfg, prep, x, W1, att_src1, att_dst1, b1, W2, att_src2,
                att_dst2, b2):
    """Per-core in_maps for the SPMD program."""
    H, HID, LB = cfg.HEADS, cfg.HID, cfg.LABELS
    C1, C2 = cfg.C1, cfg.C2
    W1 = np.asarray(W1, np.float32)
    W2 = np.asarray(W2, np.float32)
    as1 = np.asarray(att_src1, np.float32)
    ad1 = np.asarray(att_dst1, np.float32)
    as2 = np.asarray(att_src2, np.float32)
    ad2 = np.asarray(att_dst2, np.float32)
    b1 = np.asarray(b1, np.float32)
    b2 = np.asarray(b2, np.float32)

    # layer 1: per-head interleave [64 cols | zero(ones)] x4, a_s, a_d
    w1i = np.zeros((cfg.IN_F, C1), np.float32)
    b1i = np.zeros(C1, np.float32)
    for h in range(H):
        w1i[:, h * (HID + 1):h * (HID + 1) + HID] = W1[:, h * HID:(h + 1) * HID]
        b1i[h * (HID + 1):h * (HID + 1) + HID] = b1[h * HID:(h + 1) * HID]
        b1i[h * (HID + 1) + HID] = 1.0
    A_s1 = np.einsum("ihc,hc->ih", W1.reshape(-1, H, HID), as1)
    A_d1 = np.einsum("ihc,hc->ih", W1.reshape(-1, H, HID), ad1)
    b1h = b1.reshape(H, HID)
    w1e = np.concatenate([w1i, A_s1, A_d1], axis=1).astype(np.float16)
    bias1_row = np.concatenate(
        [b1i, np.einsum("hc,hc->h", b1h, as1), np.einsum("hc,hc->h", b1h, ad1)]
    ).astype(np.float16)[None, :]

    # layer 2 (input = elu+1, so subtract fp16-rounded column sums via bias)
    w2i = np.zeros((cfg.D1, C2), np.float32)
    b2i = np.zeros(C2, np.float32)
    for h in range(H):
        w2i[:, h * (LB + 1):h * (LB + 1) + LB] = W2[:, h * LB:(h + 1) * LB]
        b2i[h * (LB + 1):h * (LB + 1) + LB] = b2[h * LB:(h + 1) * LB]
        b2i[h * (LB + 1) + LB] = 1.0
    A_s2 = np.einsum("ihc,hc->ih", W2.reshape(-1, H, LB), as2)
    A_d2 = np.einsum("ihc,hc->ih", W2.reshape(-1, H, LB), ad2)
    b2h = b2.reshape(H, LB)
    w2e = np.concatenate([w2i, A_s2, A_d2], axis=1).astype(np.float16)
    w2e_f = w2e.astype(np.float32)
    bias2_row = (np.concatenate(
        [b2i, np.einsum("hc,hc->h", b2h, as2), np.einsum("hc,hc->h", b2h, ad2)]
    ) - w2e_f.sum(axis=0)).astype(np.float16)[None, :]

    iota16 = np.tile(np.arange(P, dtype=np.float16), (P, 1))
    ident = np.eye(P, dtype=np.float32).astype(np.float16)
    ones1 = np.ones((1, P), np.float16)

    # global position-ordered xT, then per-core rotation
    x = np.asarray(x, np.float32)
    xg = np.zeros((cfg.NPOS, cfg.IN_F), np.float32)
    xg[prep.gpos] = x
    xTg = np.ascontiguousarray(xg.T).astype(np.float16)

    in_maps = []
    for c in range(NCORES):
        xTc = np.ascontiguousarray(np.roll(xTg, -c * cfg.own_pad, axis=1))
        in_maps.append({
            "xT": xTc,
            "w1e": w1e, "w2e": w2e,
            "bias1": bias1_row, "bias2": bias2_row,
            "ones1": ones1, "iota16": iota16, "ident": ident,
            "is1": prep.idx_src[(1, c)], "dl1": prep.dstloc[(1, c)],
            "is2": prep.idx_src[(2, c)], "dl2": prep.dstloc[(2, c)],
        })
    return in_maps, True


def assemble_output(cfg, prep, results):
    big = np.concatenate([results[c]["out"] for c in range(NCORES)], axis=0)
    return np.ascontiguousarray(big[prep.gpos]).astype(np.float32)


_CACHE = {}


def _get_program(cfg, prep, with_bias1=True):
    key = (cfg.N, cfg.IN_F, cfg.HEADS, cfg.HID, cfg.LABELS,
           tuple(prep.caps[1][0]), tuple(prep.caps[1][1]),
           tuple(prep.caps[2][0]), tuple(prep.caps[2][1]))
    if key not in _CACHE:
        _CACHE[key] = build_program(cfg, prep)
    return _CACHE[key]


def kernel(x, edge_index, W1, att_src1, att_dst1, b1, W2, att_src2, att_dst2,
           b2):
    x = np.asarray(x)
    cfg = Cfg(n_nodes=x.shape[0], in_f=x.shape[1],
              hid=np.asarray(att_src1).shape[1], heads=np.asarray(att_src1).shape[0],
              labels=np.asarray(att_src2).shape[1])
    prep = HostPrep(cfg, np.asarray(edge_index))
    in_maps, with_bias1 = make_inputs(cfg, prep, x, W1, att_src1, att_dst1,
                                      b1, W2, att_src2, att_dst2, b2)
    nc = _get_program(cfg, prep, with_bias1)
    res = run_bass_kernel_spmd(nc, in_maps, core_ids=list(range(NCORES)))
    return assemble_output(cfg, prep, res.results)
